# revision 1
# baseline (speedup 1.0000x reference)
"""Trainium2 Bass kernel for nn_AttnBFAN (batched attention w/ focal re-norm).

Data-parallel over the batch dim: 128 batches sharded 16-per-core across 8
NeuronCores. Per batch (Q=128, C=1024, D=1024):
    attn = leaky_relu(context @ query^T, 0.1)          (C, Q)
    attn = attn / (||attn||_2 over q)                  l2norm per (b, c)
    p    = softmax(20 * attn^T, axis=c)                (Q, C)
    t    = (p > mean_c p) * p ; re_attn = t / sum_c t
    wcontext = re_attn @ context                       (Q, D)
returns (query, wcontext, re_attn).

v8: bf16 matmul path + two-level software pipelining.
 - Host pre-casts context/query to bf16 (halves HBM traffic; rel err vs
   fp32 reference ~6e-3), pre-transposes query to [d, q] chunks, and
   pre-tiles context so each partition's DMA line is 16 KB contiguous.
 - All PE transposes and bmms run in bf16 at 1.0 cycle/row. The l2-norm
   / softmax / focal chain stays f32 (per-c-column norm errors don't
   cancel in the softmax).
 - bmm2 multiplies the unnormalized focal weights t (bf16) and folds
   the 1/sum_c(t) renorm into the PSUM eviction (per-partition scale).
 - PE stream per batch b: T4-7(b+1) | ones(b) | bmm1(b+1) | T0-3(b+2) |
   t^T(b) | bmm2(b). The next batch's bmm1 and the batch-after-next's
   first transposes fill the softmax-chain latency so the PE never
   idles (and stays at the 2.4 GHz p-state). PSUM: 3-deep single-bank
   ring for ctx^T staging, 1 bank for t^T, 2+2 banks for the bmms (the
   l2 sums share the bmm2 banks so bmm1(b+1) can take a0/a1 right
   after the Prelu eviction).
"""

import os
import numpy as np
import ml_dtypes

import concourse.bacc as bacc
import concourse.mybir as mybir
import concourse.tile as tile
from concourse.bass_utils import run_bass_kernel_spmd
from concourse.masks import make_identity
from concourse.hw_specs import get_activation_tables

F32 = mybir.dt.float32
F32R = mybir.dt.float32r
BF16 = mybir.dt.bfloat16
AX = mybir.AxisListType
ALU = mybir.AluOpType
ACTF = mybir.ActivationFunctionType

NCORES = 8
NB = 128          # total batches
BPC = NB // NCORES  # batches per core
Q = 128
C = 1024
D = 1024
SMOOTH = 20.0

_CACHE = {}
STAGES = []  # (label, first_instruction_id) build-time markers for tracing


def _build():
    nc = bacc.Bacc("TRN2", target_bir_lowering=False, debug=False,
                   num_devices=NCORES, name="attn_bfan")

    def mark(label):
        STAGES.append((label, int(nc.get_next_instruction_name().split("-")[1])))

    # query pre-transposed+tiled on host: [b, p(=d%128), jd, q] bf16
    q_in = nc.dram_tensor("query", [BPC, 128, 8, Q], BF16, kind="ExternalInput")
    # context pre-tiled on host: [b, p(=c%128), jc, d] bf16
    c_in = nc.dram_tensor("context", [BPC, 128, 8, D], BF16, kind="ExternalInput")
    re_out = nc.dram_tensor("re_attn", [BPC, Q, C], F32, kind="ExternalOutput")
    wc_out = nc.dram_tensor("wcontext", [BPC, Q, D], F32, kind="ExternalOutput")

    with tile.TileContext(nc) as tc:
        with (
            tc.tile_pool(name="singles", bufs=1) as singles,
            tc.tile_pool(name="ctxp", bufs=4) as ctxp,
            tc.tile_pool(name="ctxtp", bufs=2) as ctxtp,
            tc.tile_pool(name="qTp", bufs=3) as qTp,
            tc.tile_pool(name="tTp", bufs=2) as tTp,
            tc.tile_pool(name="work", bufs=2) as work,
            tc.tile_pool(name="w1", bufs=1) as w1,
            tc.tile_pool(name="tpool", bufs=2) as tpool,
            tc.tile_pool(name="stat", bufs=2) as stat,
            tc.tile_pool(name="ps_a", bufs=1, space="PSUM") as ps_a,
            tc.tile_pool(name="ps_w", bufs=1, space="PSUM") as ps_w,
            tc.tile_pool(name="ps_f", bufs=1, space="PSUM") as ps_f,
            tc.tile_pool(name="ps_tp", bufs=3, space="PSUM") as ps_tp,
        ):
            tab_names = list(get_activation_tables("gen3").keys())
            nc.scalar.add_instruction(mybir.InstLoadActFuncSet(
                name=nc.get_next_instruction_name(),
                act_func_set_id=tab_names.index("natural_log_exp_and_others"),
                ins=[], outs=[]))
            ident = singles.tile([128, 128], F32, tag="ident")
            make_identity(nc, ident[:])
            identb = singles.tile([128, 128], BF16, tag="identb")
            nc.vector.tensor_copy(identb[:], ident[:])
            ones_f = singles.tile([128, 128], F32, tag="ones_f")
            nc.vector.memset(ones_f[:], 1.0)
            ones_r = singles.tile([128, 128], F32R, tag="ones_r")
            nc.vector.tensor_copy(ones_r[:], ones_f[:])
            ln20 = singles.tile([128, 1], F32, tag="ln20")
            nc.vector.memset(ln20[:], float(np.log(SMOOTH)))
            invC = singles.tile([128, 1], F32, tag="invC")
            nc.vector.memset(invC[:], 1.0 / C)

            ctx_t = [None] * (BPC + 3)   # plain ctx bf16 [128, 8jc, 1024d]
            ctxT_t = [None] * (BPC + 3)  # ctx^T bf16 [128, 8jd, 1024c]
            qT_t = [None] * (BPC + 3)    # q^T bf16 [128, 8jd, 128q]

            def load_batch(b):
                ctx = ctxp.tile([128, 8, D], BF16, tag="ctx", name="ctx")
                nc.gpsimd.dma_start(out=ctx[:, 0:6, :], in_=c_in[b][:, 0:6, :])
                nc.sync.dma_start(out=ctx[:, 6:8, :], in_=c_in[b][:, 6:8, :])
                ctx_t[b] = ctx
                qT = qTp.tile([128, 8, Q], BF16, tag="qT", name="qT")
                nc.sync.dma_start(out=qT[:], in_=q_in[b])
                qT_t[b] = qT

            def transpose_jd_pe(b, jd, pool=None):
                # PE-transpose ctx d-chunk jd into a 1-bank PSUM tile
                ctx = ctx_t[b]
                if ctxT_t[b] is None:
                    ctxT_t[b] = ctxtp.tile([128, 8, C], BF16, tag="ctxT",
                                           name="ctxT")
                tp = (pool or ps_tp).tile([128, 8, 128], BF16, tag="tp",
                                          name="tp")
                for jc in range(8):
                    nc.tensor.transpose(
                        tp[:, jc, :],
                        ctx[:, jc, jd * 128:(jd + 1) * 128], identb[:])
                return tp

            def copy_jd(b, jd, tp, copy_eng):
                # evict one transposed d-chunk: 1024-elem bf16 PSUM->SBUF copy
                src = tp[:].rearrange("p a b -> p (a b)")
                if copy_eng == "act":
                    nc.scalar.copy(ctxT_t[b][:, jd, :], src)
                else:
                    nc.vector.tensor_copy(ctxT_t[b][:, jd, :], src)

            def transpose_jd(b, jd, copy_eng, pool=None):
                copy_jd(b, jd, transpose_jd_pe(b, jd, pool), copy_eng)

            def bmm1(b):
                # attn^T (q, c) accumulated over 8 d-chunks -> a0/a1
                a0 = ps_a.tile([128, 512], F32, tag="a0", name="a0")
                a1 = ps_a.tile([128, 512], F32, tag="a1", name="a1")
                qT = qT_t[b]
                ctxT = ctxT_t[b]
                for jd in range(8):
                    st, sp = jd == 0, jd == 7
                    nc.tensor.matmul(a0[:], qT[:, jd, :], ctxT[:, jd, 0:512],
                                     start=st, stop=sp)
                    nc.tensor.matmul(a1[:], qT[:, jd, :], ctxT[:, jd, 512:1024],
                                     start=st, stop=sp)
                return a0, a1

            # ---- prologue: batch 0 fully staged, 1 mostly-transposed.
            # ctx(0) split across all three queues to cut first-batch latency
            ctx0 = ctxp.tile([128, 8, D], BF16, tag="ctx", name="ctx")
            nc.gpsimd.dma_start(out=ctx0[:, 0:3, :], in_=c_in[0][:, 0:3, :])
            nc.sync.dma_start(out=ctx0[:, 3:6, :], in_=c_in[0][:, 3:6, :])
            nc.scalar.dma_start(out=ctx0[:, 6:8, :], in_=c_in[0][:, 6:8, :])
            ctx_t[0] = ctx0
            qT0 = qTp.tile([128, 8, Q], BF16, tag="qT", name="qT")
            nc.scalar.dma_start(out=qT0[:], in_=q_in[0])
            qT_t[0] = qT0
            load_batch(1)
            load_batch(2)
            # bmm1(0) needs only batch 0's transposes; batch 1's follow so
            # iteration 0's chain starts as early as possible
            for jd in range(8):
                transpose_jd(0, jd, "vec" if jd % 4 != 3 else "act")
            a_cur = bmm1(0)
            for jd in range(5):
                transpose_jd(1, jd, "vec" if jd % 4 != 3 else "act")

            h0, h1 = slice(0, 512), slice(512, 1024)

            def emit_chain(b, a0, a1, spool, stags, mid=None):
                # Prelu -> sq -> [mid hook] -> ones(S into spool) -> ln/exp
                # -> softmax -> focal -> re. Returns (t, rinv).
                mark(f'i{b}_prelu')
                attn = work.tile([128, C], F32, tag="attn", name="attn")
                nc.scalar.activation(attn[:, h0], a0[:], ACTF.Prelu,
                                     bias=0.0, scale=1.0, alpha=0.1)
                nc.scalar.activation(attn[:, h1], a1[:], ACTF.Prelu,
                                     bias=0.0, scale=1.0, alpha=0.1)
                sq = w1.tile([128, C], F32R, tag="w1a", name="sq")
                nc.vector.tensor_mul(sq[:, h0], attn[:, h0], attn[:, h0])
                nc.vector.tensor_mul(sq[:, h1], attn[:, h1], attn[:, h1])
                if mid is not None:
                    mid()
                # l2 norm: ones-matmul sums over q and broadcasts
                mark(f'i{b}_ones')
                s0 = spool.tile([128, 512], F32, tag=stags[0], name="s0")
                s1 = spool.tile([128, 512], F32, tag=stags[1], name="s1")
                nc.tensor.matmul(s0[:], ones_r[:], sq[:, h0], start=True, stop=True)
                nc.tensor.matmul(s1[:], ones_r[:], sq[:, h1], start=True, stop=True)
                # 20/sqrt(S) = exp(-0.5*ln(S) + ln 20), half-split ACT/DVE
                mark(f'i{b}_ln')
                lnS = w1.tile([128, C], F32, tag="w1b", name="lnS")
                nc.scalar.activation(lnS[:, h0], s0[:], ACTF.Ln)
                nc.scalar.activation(lnS[:, h1], s1[:], ACTF.Ln)
                rn20 = w1.tile([128, C], F32, tag="w1c", name="rn20")
                u = w1.tile([128, C], F32, tag="w1a", name="u")
                pu = work.tile([128, C], F32, tag="pu", name="pu")
                rs0 = stat.tile([128, 1], F32, tag="rs0", name="rs0")
                rs1 = stat.tile([128, 1], F32, tag="rs1", name="rs1")
                nc.scalar.activation(rn20[:, h0], lnS[:, h0], ACTF.Exp,
                                     bias=ln20[:], scale=-0.5)
                nc.scalar.activation(rn20[:, h1], lnS[:, h1], ACTF.Exp,
                                     bias=ln20[:], scale=-0.5)
                nc.vector.tensor_mul(u[:, h0], attn[:, h0], rn20[:, h0])
                nc.vector.tensor_mul(u[:, h1], attn[:, h1], rn20[:, h1])
                nc.scalar.activation(pu[:, h0], u[:, h0], ACTF.Exp,
                                     bias=0.0, scale=1.0, accum_out=rs0[:])
                nc.scalar.activation(pu[:, h1], u[:, h1], ACTF.Exp,
                                     bias=0.0, scale=1.0, accum_out=rs1[:])
                # thr = (rs0 + rs1) / C in one DVE op
                thr = stat.tile([128, 1], F32, tag="thr", name="thr")
                nc.vector.scalar_tensor_tensor(
                    out=thr[:], in0=rs0[:], scalar=rs1[:], in1=invC[:],
                    op0=ALU.add, op1=ALU.mult)
                # focal: t = (pu > thr) * pu (bf16), half-split so the t^T
                # transposes pipeline behind stt-h0; rinv only gates evictions
                mark(f'i{b}_focal')
                t = tpool.tile([128, C], BF16, tag="t", name="t")
                ts0 = stat.tile([128, 1], F32, tag="ts0", name="ts0")
                ts1 = stat.tile([128, 1], F32, tag="ts1", name="ts1")
                nc.vector.scalar_tensor_tensor(
                    out=t[:, h0], in0=pu[:, h0], scalar=thr[:], in1=pu[:, h0],
                    op0=ALU.is_gt, op1=ALU.mult, accum_out=ts0[:])
                nc.vector.scalar_tensor_tensor(
                    out=t[:, h1], in0=pu[:, h1], scalar=thr[:], in1=pu[:, h1],
                    op0=ALU.is_gt, op1=ALU.mult, accum_out=ts1[:])
                return t, ts0, ts1

            def emit_tail(b, t, ts0, ts1, late=None):
                # t^T transposes (half-pipelined), late ctx^T copies, bmm2,
                # wc eviction scaled by rinv
                mark(f'i{b}_tT')
                tT = tTp.tile([128, 8, Q], BF16, tag="tT", name="tT")
                tpf = ps_f.tile([128, 8, 128], BF16, tag="tpf", name="tpf")
                for jc in range(8):
                    nc.tensor.transpose(
                        tpf[:, jc, :],
                        t[:, jc * 128:(jc + 1) * 128], identb[:])
                    if jc == 3:
                        nc.vector.tensor_copy(
                            tT[:, 0:4, :].rearrange("p a b -> p (a b)"),
                            tpf[:, 0:4, :].rearrange("p a b -> p (a b)"))
                nc.vector.tensor_copy(
                    tT[:, 4:8, :].rearrange("p a b -> p (a b)"),
                    tpf[:, 4:8, :].rearrange("p a b -> p (a b)"))
                # renorm + re eviction moved behind the tT copies: neither
                # gates bmm2, so the PE reaches it sooner
                ts = stat.tile([128, 1], F32, tag="ts", name="ts")
                nc.vector.tensor_add(ts[:], ts0[:], ts1[:])
                rinv = stat.tile([128, 1], F32, tag="rinv", name="rinv")
                nc.vector.reciprocal(rinv[:], ts[:])
                if late is not None:
                    late()
                re = work.tile([128, C], F32, tag="re", name="re")
                nc.scalar.activation(re[:], t[:], ACTF.Copy, bias=0.0,
                                     scale=rinv[:])
                if b == BPC - 1:
                    nc.scalar.dma_start(out=re_out[b][:, 0:512],
                                        in_=re[:, 0:512])
                    nc.gpsimd.dma_start(out=re_out[b][:, 512:1024],
                                        in_=re[:, 512:1024])
                else:
                    nc.scalar.dma_start(out=re_out[b], in_=re[:])
                mark(f'i{b}_bmm2')
                ctx = ctx_t[b]
                w0 = ps_w.tile([128, 512], F32, tag="w0", name="w0")
                w2 = ps_w.tile([128, 512], F32, tag="w2", name="w2")
                for jc in range(8):
                    st, sp = jc == 0, jc == 7
                    nc.tensor.matmul(w0[:], tT[:, jc, :], ctx[:, jc, 0:512],
                                     start=st, stop=sp)
                    nc.tensor.matmul(w2[:], tT[:, jc, :], ctx[:, jc, 512:1024],
                                     start=st, stop=sp)
                mark(f'i{b}_wc')
                wc = work.tile([128, D], F32, tag="wc", name="wc")
                nc.scalar.activation(wc[:, h0], w0[:], ACTF.Copy,
                                     bias=0.0, scale=rinv[:])
                nc.scalar.activation(wc[:, h1], w2[:], ACTF.Copy,
                                     bias=0.0, scale=rinv[:])
                if b == BPC - 1:
                    # last batch: fan the store across the now-idle queues
                    wcd = wc_out[b].rearrange("q (g e) -> q g e", g=4)
                    wcs = wc[:].rearrange("q (g e) -> q g e", g=4)
                    nc.sync.dma_start(out=wcd[:, 0:2, :], in_=wcs[:, 0:2, :])
                    nc.gpsimd.dma_start(out=wcd[:, 2:3, :], in_=wcs[:, 2:3, :])
                    nc.scalar.dma_start(out=wcd[:, 3:4, :], in_=wcs[:, 3:4, :])
                else:
                    nc.sync.dma_start(out=wc_out[b], in_=wc[:])

            for b in range(BPC - 1):  # batch BPC-1 is merged into BPC-2
                a0, a1 = a_cur
                mark(f'iter{b}')
                if b + 3 < BPC:
                    load_batch(b + 3)

                def mid_tp(b=b):
                    # finish next batch's ctx^T (jd 5-7) in the pre-Ln slack
                    mark(f'i{b}_tp57')
                    transpose_jd(b + 1, 5, "vec")
                    transpose_jd(b + 1, 6, "act")
                    transpose_jd(b + 1, 7, "vec")

                t, ts0c, ts1c = emit_chain(b, a0, a1, ps_w, ("w0", "w2"),
                                           mid=mid_tp)

                # PE: bmm1(b+1) + first transposes of b+2 fill the chain tail
                mark(f'i{b}_bmm1n')
                a_cur = bmm1(b + 1)
                tp_late = []
                if b + 2 < BPC:
                    mark(f'i{b}_tp04')
                    transpose_jd(b + 2, 0, "act")
                    tp_late.append((1, transpose_jd_pe(b + 2, 1)))
                    transpose_jd(b + 2, 2, "act")
                    tp_late.append((3, transpose_jd_pe(b + 2, 3)))
                    tp_late.append((4, transpose_jd_pe(b + 2, 4)))

                def late(b=b, tp_late=tp_late):
                    for jd, tp in tp_late:
                        copy_jd(b + 2, jd, tp, "vec")

                if b == BPC - 2:
                    # epilogue: interleave the last batch's chain (its l2 sums
                    # use the freed a-banks); batch b's tail fills its latency
                    na0, na1 = a_cur
                    t15, ts015, ts115 = emit_chain(
                        BPC - 1, na0, na1, ps_a, ("a0", "a1"),
                        mid=lambda: emit_tail(b, t, ts0c, ts1c, late=late))
                    emit_tail(BPC - 1, t15, ts015, ts115)
                else:
                    emit_tail(b, t, ts0c, ts1c, late=late)
                ctx_t[b] = None
                ctxT_t[b] = None
                qT_t[b] = None

    nc.compile()
    return nc


def kernel(query: np.ndarray, context: np.ndarray):
    query = np.ascontiguousarray(query, dtype=np.float32)
    context = np.ascontiguousarray(context, dtype=np.float32)
    assert query.shape == (NB, Q, D) and context.shape == (NB, C, D)

    if "nc" not in _CACHE:
        _CACHE["nc"] = _build()
    nc = _CACHE["nc"]

    bf16 = ml_dtypes.bfloat16
    # qT host prep: (B, Q, D) -> [b, p, jd, q] where d = jd*128 + p
    qT = np.ascontiguousarray(
        query.transpose(0, 2, 1).reshape(NB, 8, 128, Q).transpose(0, 2, 1, 3)
    ).astype(bf16)
    # context: (B, C, D) -> [b, p, jc, d] with c = jc*128 + p
    ctx_bf = np.ascontiguousarray(
        context.reshape(NB, 8, 128, D).transpose(0, 2, 1, 3)
    ).astype(bf16)

    in_maps = []
    for k in range(NCORES):
        sl = slice(k * BPC, (k + 1) * BPC)
        in_maps.append({"query": qT[sl], "context": ctx_bf[sl]})

    trace = os.environ.get("KERNEL_TRACE", "0") == "1"
    res = run_bass_kernel_spmd(nc, in_maps, core_ids=list(range(NCORES)),
                               trace=trace)
    _CACHE["last_res"] = res

    re_attn = np.concatenate([r["re_attn"] for r in res.results], axis=0)
    wcontext = np.concatenate([r["wcontext"] for r in res.results], axis=0)
    return query, wcontext, re_attn

